# revision 41
# baseline (speedup 1.0000x reference)
"""MLA (multi-head latent attention) prefill kernel for 8 Trainium2 NeuronCores.

Sharding: pure data-parallel over (batch, query-chunk). Core c handles batch
c//4 and query rows [512*(c%4), 512*(c%4+1)). The KV path (latent + k_rope,
full 2048 keys) is recomputed per core from the batch's hidden states
(~9% FLOP overhead) so there are ZERO collectives; every core writes a
disjoint [512, 2048] slice of the output.

Structure (final):
 - De-absorbed prefill math: k_nope[h] = kv_lat @ k_up[h]^T and
   v_lat[h] = kv_lat @ v_up[h]^T materialized per head(-group): scores
   contract 128+64 channels (not 512+64), attn rank 128 (not 512).
 - All matmuls bf16 (fp32 PSUM) for the PE fast-weight-load path.
 - LN gammas folded into consuming weights, betas into eviction biases;
   LN itself is one fused (x-mu)*rstd vector op.
 - Phase 1 is software-pipelined in segments: all projection matmuls run
   back-to-back while LayerNorms trail on vector/scalar; transposes are
   batched (5 per PSUM bank, single strided eviction) and interleaved
   with the group-0 attention prologue so the PE never waits on LN.
 - RoPE score matmuls (K=64) packed pairwise onto disjoint PE row groups.
 - Softmax denominator: probs folded pair->quad->oct on vector, two
   ones-matmuls per head; fast approximate reciprocal.
 - attn_v^T stays in SBUF as the o_proj lhsT; o_proj runs as 4 quarter
   passes with double-buffered PSUM so evictions overlap matmuls.
"""

import math
from contextlib import ExitStack

import numpy as np
from ml_dtypes import bfloat16

import concourse.bass as bass
import concourse.tile as tile
from concourse import bacc, mybir
from concourse.bass_utils import run_bass_kernel_spmd
from concourse.masks import make_identity

F32 = mybir.dt.float32
F32R = mybir.dt.float32r
BF16 = mybir.dt.bfloat16
AF = mybir.ActivationFunctionType
OP = mybir.AluOpType

B, S, D = 2, 2048, 2048
H = 16
LAT = 1536
R = 512
DN, DR, DV = 128, 64, 128
EPS = 1e-5
SCALE = 1.0 / math.sqrt(DN + DR)

P = 128
CH = 512
NQT = CH // P
NKT = S // P
NDT = D // P
NLT = LAT // P

N_CORES = 8


def _bcast_rows(t, n, length):
    return bass.AP(tensor=t, offset=0, ap=[[0, n], [1, length]])


def build_nc():
    nc = bacc.Bacc(None, target_bir_lowering=False)

    hst4 = nc.dram_tensor("hst4", [NKT // 2 - 2, NDT, 2, P, P], BF16,
                          kind="ExternalInput")
    hsqt = nc.dram_tensor("hsqt", [2, NDT, 2, P, P], BF16, kind="ExternalInput")
    wqa_t = nc.dram_tensor("wqa_t", [D, LAT], BF16, kind="ExternalInput")
    wqb_t = nc.dram_tensor("wqb_t", [LAT, H * DN], BF16, kind="ExternalInput")
    wqr_t = nc.dram_tensor("wqr_t", [LAT, H * DR], BF16, kind="ExternalInput")
    wkva_t = nc.dram_tensor("wkva_t", [D, R + DR], BF16, kind="ExternalInput")
    kup_t = nc.dram_tensor("kup_t", [R, H * DN], BF16, kind="ExternalInput")
    vup_t = nc.dram_tensor("vup_t", [R, H * DV], BF16, kind="ExternalInput")
    wo_t = nc.dram_tensor("wo_t", [H * DV, D], BF16, kind="ExternalInput")
    bqn_v = nc.dram_tensor("bqn_v", [H * DN], F32, kind="ExternalInput")
    bqr_v = nc.dram_tensor("bqr_v", [H * DR], F32, kind="ExternalInput")
    bkn_v = nc.dram_tensor("bkn_v", [H * DN], F32, kind="ExternalInput")
    bvv_v = nc.dram_tensor("bvv_v", [H * DV], F32, kind="ExternalInput")
    ones_in = nc.dram_tensor("ones_in", [P, P], F32R, kind="ExternalInput")
    ck_tab = nc.dram_tensor("ck_tab", [S, DR // 2], F32, kind="ExternalInput")
    sk_tab = nc.dram_tensor("sk_tab", [S, DR // 2], F32, kind="ExternalInput")
    cq_tab = nc.dram_tensor("cq_tab", [P, CH], F32, kind="ExternalInput")
    sq_tab = nc.dram_tensor("sq_tab", [P, CH], F32, kind="ExternalInput")
    out_c = nc.dram_tensor("out_c", [CH, D], F32, kind="ExternalOutput")

    with tile.TileContext(nc) as tc, ExitStack() as octx:
        res = octx.enter_context(tc.tile_pool(name="res", bufs=1))
        # k_full^T: 4x128 latent planes; plane 4 = rope channels duplicated
        # to partitions 64:128 for row-group packing of the rope matmuls.
        kfull = res.tile([P, 5, S], BF16)
        qlat_t = res.tile([P, NLT, CH], BF16)
        kupT = res.tile([P, 4, H * DN], BF16)
        vupT = res.tile([P, 4, H * DV], BF16)

        consts = octx.enter_context(tc.tile_pool(name="consts", bufs=1))
        ident = consts.tile([P, P], BF16)
        make_identity(nc, ident)
        ones_t = consts.tile([P, P], F32R)
        eps_t = consts.tile([P, 1], F32)
        nc.vector.memset(eps_t, EPS)
        cq_t = consts.tile([P, CH], F32)
        sq_t = consts.tile([P, CH], F32)
        bqn_t = consts.tile([P, H], F32)
        bqr_t = consts.tile([P, 8], F32)
        bkn_t = consts.tile([P, H], F32)
        bvv_bc = consts.tile([P, H * DV], F32)

        wop = octx.enter_context(tc.tile_pool(name="wop", bufs=6))

        # group-0 attention prologue tiles (built during phase 1 segment C)
        qwork = octx.enter_context(tc.tile_pool(name="qwork", bufs=1))
        gwork = octx.enter_context(tc.tile_pool(name="gwork", bufs=1))

        with ExitStack() as p1all:
            wkvp = p1all.enter_context(tc.tile_pool(name="wkvp", bufs=1))
            hsqp = p1all.enter_context(tc.tile_pool(name="hsqp", bufs=1))
            hsl = p1all.enter_context(tc.tile_pool(name="hsl", bufs=4))
            hk_pre = []
            lnsp = p1all.enter_context(tc.tile_pool(name="lnsp", bufs=1))
            qln_all = lnsp.tile([P, NQT, LAT], BF16)
            lnf_all = lnsp.tile([P, NKT, R], BF16)
            kro_all = lnsp.tile([P, NKT, P], BF16)

            # ================= segment A: q-mix matmuls =================
            with ExitStack() as pA:
                wqap = pA.enter_context(tc.tile_pool(name="wqap", bufs=2))
                mixp = pA.enter_context(tc.tile_pool(name="mixp", bufs=1))
                lnp = pA.enter_context(tc.tile_pool(name="lnp", bufs=2))
                psA = pA.enter_context(tc.tile_pool(name="psA", bufs=1, space="PSUM"))

                qmix_all = mixp.tile([P, NQT, LAT], BF16)
                wkv_all = wkvp.tile([P, NDT, R + DR], BF16)
                ck_t = wkvp.tile([P, NKT, DR // 2], F32)
                sk_t = wkvp.tile([P, NKT, DR // 2], F32)

                # heavy phase-1b/2 weight DMAs are spread through the j/dt
                # loop so they never serialize ahead of the q-mix operands
                def _stream_weights(j, dt):
                    if j == 0 and dt % 4 == 2:
                        a = dt // 4
                        nc.sync.dma_start(
                            wkv_all[:, 4 * a:4 * (a + 1), :],
                            wkva_t[512 * a:512 * (a + 1), :].rearrange(
                                "(t p) c -> p t c", p=P
                            ),
                        )
                    elif j == 0 and dt == 13:
                        nc.sync.dma_start(
                            ck_t[:], ck_tab.ap().rearrange("(t p) j -> p t j", p=P)
                        )
                        nc.sync.dma_start(
                            sk_t[:], sk_tab.ap().rearrange("(t p) j -> p t j", p=P)
                        )
                    elif j == 1 and dt % 4 == 2:
                        rc = dt // 4
                        nc.sync.dma_start(
                            kupT[:, rc, :], kup_t[rc * P:(rc + 1) * P, :]
                        )
                    elif j == 2 and dt % 4 == 2:
                        rc = dt // 4
                        nc.sync.dma_start(
                            vupT[:, rc, :], vup_t[rc * P:(rc + 1) * P, :]
                        )
                    elif j == 1 and dt == 13:
                        nc.sync.dma_start(cq_t[:], cq_tab[:, :])
                        nc.sync.dma_start(sq_t[:], sq_tab[:, :])
                    elif j == 2 and dt in (1, 5):
                        a2 = dt // 4
                        hk8 = hsl.tile([P, 4, 2, P], BF16, tag="hs",
                                       name="hkpre")
                        nc.sync.dma_start(
                            hk8[:],
                            hst4[0, 4 * a2:4 * (a2 + 1)].rearrange(
                                "d k p c -> p d k c"
                            ),
                        )
                        hk_pre.append(hk8)
                    elif j == 2 and dt == 15:
                        nc.sync.dma_start(ones_t[:], ones_in[:, :])
                        nc.sync.dma_start(
                            bqn_t[:], bqn_v.ap().rearrange("(h p) -> p h", p=P)
                        )
                        nc.sync.dma_start(
                            bqr_t[:], bqr_v.ap().rearrange("(a p) -> p a", p=P)
                        )
                        nc.sync.dma_start(
                            bkn_t[:], bkn_v.ap().rearrange("(h p) -> p h", p=P)
                        )
                        nc.sync.dma_start(bvv_bc[:], _bcast_rows(bvv_v, P, H * DV))

                hsq_all = hsqp.tile([P, 2, NDT, 2, P], BF16)
                for pair in range(2):
                    nc.sync.dma_start(
                        hsq_all[:, pair, 0:1, :, :],
                        hsqt[pair, 0:1].rearrange("d k p c -> p d k c"),
                    )
                for pair in range(2):
                    nc.sync.dma_start(
                        hsq_all[:, pair, 1:4, :, :],
                        hsqt[pair, 1:4].rearrange("d k p c -> p d k c"),
                    )
                for a in range(1, 4):
                    for pair in range(2):
                        nc.sync.dma_start(
                            hsq_all[:, pair, 4 * a:4 * (a + 1), :, :],
                            hsqt[pair, 4 * a:4 * (a + 1)].rearrange(
                                "d k p c -> p d k c"
                            ),
                        )
                for j in range(3):
                    pqj = psA.tile([P, NQT, 512], F32, tag="pq", bufs=2)
                    for a in range(4):
                        wqa_c = wqap.tile([P, 4, 512], BF16, tag="wqa")
                        if j == 0 and a == 0:
                            nc.sync.dma_start(
                                wqa_c[:, 0:1, :],
                                wqa_t[0:P, 0:512].rearrange(
                                    "(t p) c -> p t c", p=P
                                ),
                            )
                            nc.sync.dma_start(
                                wqa_c[:, 1:4, :],
                                wqa_t[P:512, 0:512].rearrange(
                                    "(t p) c -> p t c", p=P
                                ),
                            )
                        else:
                            nc.sync.dma_start(
                                wqa_c[:],
                                wqa_t[a * 512:(a + 1) * 512,
                                      j * 512:(j + 1) * 512].rearrange(
                                    "(t p) c -> p t c", p=P
                                ),
                            )
                        for i in range(4):
                            dt = 4 * a + i
                            for qt in range(NQT):
                                nc.tensor.matmul(
                                    pqj[:, qt, :],
                                    hsq_all[:, qt // 2, dt, qt % 2, :],
                                    wqa_c[:, i, :],
                                    start=(dt == 0), stop=(dt == NDT - 1),
                                )
                            _stream_weights(j, dt)
                    for qt in range(NQT):
                        nc.vector.tensor_copy(
                            qmix_all[:, qt, j * 512:(j + 1) * 512], pqj[:, qt, :]
                        )

                # q LayerNorms (vector trails; consumed by segment C)
                for qt in range(NQT):
                    statsq = lnp.tile([P, 3, 6], F32, tag="statsq")
                    for j in range(3):
                        nc.vector.bn_stats(
                            statsq[:, j, :], qmix_all[:, qt, j * 512:(j + 1) * 512]
                        )
                    mvq = lnp.tile([P, 2], F32, tag="mv")
                    nc.vector.bn_aggr(mvq[:], statsq[:])
                    rstdq = lnp.tile([P, 1], F32, tag="rstd")
                    nc.scalar.activation(
                        rstdq[:], mvq[:, 1:2], AF.Sqrt, bias=eps_t[:]
                    )
                    nc.vector.reciprocal(rstdq[:], rstdq[:])
                    nc.vector.tensor_scalar(
                        qln_all[:, qt, :], qmix_all[:, qt, :], mvq[:, 0:1],
                        rstdq[:], op0=OP.subtract, op1=OP.mult,
                    )

            # ============= segment B: kv-mix matmuls + trailing LN =============
            with ExitStack() as pB:
                mixp = pB.enter_context(tc.tile_pool(name="mixp", bufs=3))
                lnp = pB.enter_context(tc.tile_pool(name="lnp", bufs=2))
                psB = pB.enter_context(tc.tile_pool(name="psB", bufs=1, space="PSUM"))

                for ktp in range(NKT // 2):
                    kt0 = 2 * ktp
                    pm = [
                        psB.tile([P, 2, 512], F32, tag=f"pmix{i}", bufs=2,
                                 name=f"pm{i}")
                        for i in range(2)
                    ]
                    for a in range(4):
                        if ktp < 2:
                            # this kt-pair is the core's own q-chunk: tiles
                            # are already resident from segment A
                            hk8 = hsq_all[:, ktp, 4 * a:4 * (a + 1), :, :]
                        elif ktp == 2 and a < 2:
                            hk8 = hk_pre[a]
                        else:
                            hk8t = hsl.tile([P, 4, 2, P], BF16, tag="hs")
                            nc.sync.dma_start(
                                hk8t[:],
                                hst4[ktp - 2, 4 * a:4 * (a + 1)].rearrange(
                                    "d k p c -> p d k c"
                                ) if ktp > 2 else
                                hst4[0, 4 * a:4 * (a + 1)].rearrange(
                                    "d k p c -> p d k c"
                                ),
                            )
                            hk8 = hk8t
                        for di in range(4):
                            dt = 4 * a + di
                            st = (dt == 0)
                            sp = (dt == NDT - 1)
                            for i in range(2):
                                nc.tensor.matmul(
                                    pm[i][:, 0, 0:288], hk8[:, di, i, :],
                                    wkv_all[:, dt, 0:288], start=st, stop=sp,
                                )
                                nc.tensor.matmul(
                                    pm[i][:, 1, 0:288], hk8[:, di, i, :],
                                    wkv_all[:, dt, 288:576], start=st, stop=sp,
                                )
                    for i in range(2):
                        kt = kt0 + i
                        kvmix = mixp.tile([P, R + DR], F32, tag="kvmix")
                        nc.scalar.copy(kvmix[:, 0:288], pm[i][:, 0, 0:288])
                        nc.scalar.copy(kvmix[:, 288:576], pm[i][:, 1, 0:288])

                        stats = lnp.tile([P, 6], F32, tag="stats")
                        nc.vector.bn_stats(stats[:], kvmix[:, 0:R])
                        mv = lnp.tile([P, 2], F32, tag="mv")
                        nc.vector.bn_aggr(mv[:], stats[:])
                        rstd = lnp.tile([P, 1], F32, tag="rstd")
                        nc.scalar.activation(
                            rstd[:], mv[:, 1:2], AF.Sqrt, bias=eps_t[:]
                        )
                        nc.vector.reciprocal(rstd[:], rstd[:])
                        nc.vector.tensor_scalar(
                            lnf_all[:, kt, :], kvmix[:, 0:R], mv[:, 0:1],
                            rstd[:], op0=OP.subtract, op1=OP.mult,
                        )

                        # RoPE, rotated pairs duplicated to cols 64:128
                        t1 = lnp.tile([P, DR // 2], F32, tag="t1")
                        t2 = lnp.tile([P, DR // 2], F32, tag="t2")
                        x1 = kvmix[:, R:R + 32]
                        x2 = kvmix[:, R + 32:R + 64]
                        kro = kro_all[:, kt, :]
                        nc.vector.tensor_tensor(t2[:], x1, ck_t[:, kt, :], OP.mult)
                        nc.vector.tensor_tensor(t1[:], x2, sk_t[:, kt, :], OP.mult)
                        nc.vector.tensor_tensor(kro[:, 0:32], t2[:], t1[:], OP.subtract)
                        nc.vector.tensor_tensor(kro[:, 64:96], t2[:], t1[:], OP.subtract)
                        nc.vector.tensor_tensor(t2[:], x1, sk_t[:, kt, :], OP.mult)
                        nc.vector.tensor_tensor(t1[:], x2, ck_t[:, kt, :], OP.mult)
                        nc.vector.tensor_tensor(kro[:, 32:64], t2[:], t1[:], OP.add)
                        nc.vector.tensor_tensor(kro[:, 96:128], t2[:], t1[:], OP.add)

            # ==== segment C: batched transposes + group-0 attention prologue ====
            with ExitStack() as pC:
                psC = pC.enter_context(tc.tile_pool(name="psC", bufs=1, space="PSUM"))
                wqs0 = pC.enter_context(tc.tile_pool(name="wqs0", bufs=2))

                # q latent transposes: 6 per PSUM bank, one strided evict each
                ev = 0
                for qt in range(NQT):
                    for half in range(2):
                        ptq = psC.tile([P, 6, P], BF16, tag="ptr", bufs=2)
                        for i in range(6):
                            lt = half * 6 + i
                            nc.tensor.transpose(
                                ptq[:, i, :],
                                qln_all[:, qt, lt * P:(lt + 1) * P], ident[:],
                            )
                        dst = qlat_t[:, half * 6:(half + 1) * 6,
                                     qt * P:(qt + 1) * P]
                        if ev % 2 == 0:
                            nc.vector.tensor_copy(dst, ptq[:])
                        else:
                            nc.scalar.copy(dst, ptq[:])
                        ev += 1

                # group-0 q_rope projection + rotation
                qraw = qwork.tile([P, 2, CH], F32, tag="qraw")
                for half in range(2):
                    wrc = wqs0.tile([P, NLT, P], BF16, tag="wq")
                    nc.sync.dma_start(
                        wrc[:],
                        wqr_t[:, half * 512:half * 512 + P].rearrange(
                            "(t p) c -> p t c", p=P
                        ),
                    )
                    pr = psC.tile([P, 512], F32, tag="proj", bufs=2)
                    for lt in range(NLT):
                        nc.tensor.matmul(
                            pr[:], wrc[:, lt, :], qlat_t[:, lt, :],
                            start=(lt == 0), stop=(lt == NLT - 1),
                        )
                    nc.scalar.add(
                        qraw[:, half, :], pr[:], bqr_t[:, half * 4:half * 4 + 1]
                    )
                qro0 = qwork.tile([P, 2, CH], BF16, tag="qro")
                tm = qwork.tile([P, CH], F32, tag="tm")
                tn = qwork.tile([P, CH], F32, tag="tn")
                x1, x2 = qraw[:, 0, :], qraw[:, 1, :]
                nc.vector.tensor_tensor(tm[:], x2, sq_t[:], OP.mult)
                nc.vector.tensor_tensor(tn[:], x1, cq_t[:], OP.mult)
                nc.vector.tensor_tensor(qro0[:, 0, :], tn[:], tm[:], OP.subtract)
                nc.vector.tensor_tensor(tm[:], x2, cq_t[:], OP.mult)
                nc.vector.tensor_tensor(tn[:], x1, sq_t[:], OP.mult)
                nc.vector.tensor_tensor(qro0[:, 1, :], tn[:], tm[:], OP.add)

                # kv transposes (5 per bank, single evict) + group-0 v_lat
                vlatq0 = gwork.tile([P, NKT, 512], BF16, tag="vlatq", bufs=1)
                for kt in range(NKT):
                    pt = psC.tile([P, 5, P], BF16, tag="ptr", bufs=2)
                    for j in range(4):
                        nc.tensor.transpose(
                            pt[:, j, :],
                            lnf_all[:, kt, j * P:(j + 1) * P], ident[:],
                        )
                    nc.tensor.transpose(pt[:, 4, :], kro_all[:, kt, :], ident[:])
                    dst = kfull[:, 0:5, kt * P:(kt + 1) * P]
                    if kt % 2 == 0:
                        nc.vector.tensor_copy(dst, pt[:])
                    else:
                        nc.scalar.copy(dst, pt[:])

                    pv1 = psC.tile([P, 512], F32, tag="proj", bufs=2)
                    for rc in range(4):
                        nc.tensor.matmul(
                            pv1[:], kfull[:, rc, kt * P:(kt + 1) * P],
                            vupT[:, rc, 0:512],
                            start=(rc == 0), stop=(rc == 3),
                        )
                    nc.vector.tensor_tensor(
                        vlatq0[:, kt, :], pv1[:], bvv_bc[:, 0:512], OP.add
                    )

        # ====================== phase 2: attention head loop ======================
        attp = octx.enter_context(tc.tile_pool(name="attp", bufs=1))
        avT = attp.tile([P, H, CH], BF16)

        wo_pre = []
        with ExitStack() as p2:
            wqs = p2.enter_context(tc.tile_pool(name="wqs", bufs=2))
            hwork = p2.enter_context(tc.tile_pool(name="hwork", bufs=2))
            probs_p = p2.enter_context(tc.tile_pool(name="probs_p", bufs=2))
            foldp = p2.enter_context(tc.tile_pool(name="foldp", bufs=3))
            ps2 = p2.enter_context(tc.tile_pool(name="ps2", bufs=1, space="PSUM"))

            qro, vlatq = qro0, vlatq0
            for h in range(H):
                g, m = divmod(h, 4)
                if m == 0 and g > 0:
                    qraw = qwork.tile([P, 2, CH], F32, tag="qraw")
                    for half in range(2):
                        wrc = wqs.tile([P, NLT, P], BF16, tag="wq")
                        col0 = half * 512 + g * P
                        nc.sync.dma_start(
                            wrc[:],
                            wqr_t[:, col0:col0 + P].rearrange(
                                "(t p) c -> p t c", p=P
                            ),
                        )
                        pr = ps2.tile([P, 512], F32, tag="proj", bufs=2)
                        for lt in range(NLT):
                            nc.tensor.matmul(
                                pr[:], wrc[:, lt, :], qlat_t[:, lt, :],
                                start=(lt == 0), stop=(lt == NLT - 1),
                            )
                        nc.scalar.add(
                            qraw[:, half, :], pr[:],
                            bqr_t[:, half * 4 + g:half * 4 + g + 1],
                        )
                    qro = qwork.tile([P, 2, CH], BF16, tag="qro")
                    tm = qwork.tile([P, CH], F32, tag="tm")
                    tn = qwork.tile([P, CH], F32, tag="tn")
                    x1, x2 = qraw[:, 0, :], qraw[:, 1, :]
                    nc.vector.tensor_tensor(tm[:], x2, sq_t[:], OP.mult)
                    nc.vector.tensor_tensor(tn[:], x1, cq_t[:], OP.mult)
                    nc.vector.tensor_tensor(qro[:, 0, :], tn[:], tm[:], OP.subtract)
                    nc.vector.tensor_tensor(tm[:], x2, cq_t[:], OP.mult)
                    nc.vector.tensor_tensor(tn[:], x1, sq_t[:], OP.mult)
                    nc.vector.tensor_tensor(qro[:, 1, :], tn[:], tm[:], OP.add)

                    vlatq = gwork.tile([P, NKT, 512], BF16, tag="vlatq", bufs=1)
                    for kt in range(NKT):
                        pv1 = ps2.tile([P, 512], F32, tag="proj", bufs=2)
                        for rc in range(4):
                            nc.tensor.matmul(
                                pv1[:], kfull[:, rc, kt * P:(kt + 1) * P],
                                vupT[:, rc, g * 512:(g + 1) * 512],
                                start=(rc == 0), stop=(rc == 3),
                            )
                        nc.vector.tensor_tensor(
                            vlatq[:, kt, :], pv1[:],
                            bvv_bc[:, g * 512:(g + 1) * 512], OP.add,
                        )

                if h == H - 1:
                    for i in range(3):
                        wo = wop.tile([P, 512], BF16, tag="wo")
                        nc.sync.dma_start(wo[:], wo_t[i * P:(i + 1) * P, 0:512])
                        wo_pre.append(wo)

                wb = wqs.tile([P, NLT, P], BF16, tag="wq")
                nc.sync.dma_start(
                    wb[:],
                    wqb_t[:, h * P:(h + 1) * P].rearrange("(t p) c -> p t c", p=P),
                )
                pn = ps2.tile([P, 512], F32, tag="proj", bufs=2)
                for lt in range(NLT):
                    nc.tensor.matmul(
                        pn[:], wb[:, lt, :], qlat_t[:, lt, :],
                        start=(lt == 0), stop=(lt == NLT - 1),
                    )
                qnope = hwork.tile([P, CH], BF16, tag="qnope")
                nc.scalar.add(qnope[:], pn[:], bqn_t[:, h:h + 1])

                knopeT = hwork.tile([P, S], BF16, tag="knopeT")
                for kc in range(4):
                    pk = ps2.tile([P, 512], F32, tag="proj", bufs=2)
                    for rc in range(4):
                        nc.tensor.matmul(
                            pk[:], kupT[:, rc, h * P:(h + 1) * P],
                            kfull[:, rc, kc * 512:(kc + 1) * 512],
                            start=(rc == 0), stop=(rc == 3),
                        )
                    nc.scalar.add(
                        knopeT[:, kc * 512:(kc + 1) * 512], pk[:],
                        bkn_t[:, h:h + 1],
                    )

                qropeT = hwork.tile([P, CH], BF16, tag="qropeT")
                nc.sync.dma_start(qropeT[0:32, :], qro[m * 32:(m + 1) * 32, 0, :])
                nc.sync.dma_start(qropeT[32:64, :], qro[m * 32:(m + 1) * 32, 1, :])
                nc.sync.dma_start(qropeT[64:96, :], qro[m * 32:(m + 1) * 32, 0, :])
                nc.sync.dma_start(qropeT[96:128, :], qro[m * 32:(m + 1) * 32, 1, :])

                probs = probs_p.tile([P, NKT, CH], BF16, tag="probs")
                folds = []
                quads = []
                octs = []
                pv = ps2.tile([P, 512], F32, tag="attn", bufs=1)
                pd = ps2.tile([P, 512], F32, tag="den", bufs=1)
                for p in range(NKT // 2):
                    kt, kt1 = 2 * p, 2 * p + 1
                    sc = ps2.tile([P, 2, 512], F32, tag="scores", bufs=2)
                    nc.tensor.matmul(
                        sc[:, 0, :], knopeT[:, kt * P:(kt + 1) * P], qnope[:],
                        start=True, stop=False,
                    )
                    nc.tensor.matmul(
                        sc[:, 1, :], knopeT[:, kt1 * P:(kt1 + 1) * P], qnope[:],
                        start=True, stop=False,
                    )
                    nc.tensor.matmul(
                        sc[:, 0, :], kfull[0:DR, 4, kt * P:(kt + 1) * P],
                        qropeT[0:DR, :], start=False, stop=True,
                    )
                    nc.tensor.matmul(
                        sc[:, 1, :], kfull[DR:P, 4, kt1 * P:(kt1 + 1) * P],
                        qropeT[DR:P, :], start=False, stop=True,
                        tile_position=(DR, 0),
                    )
                    nc.scalar.activation(probs[:, kt:kt + 2, :], sc[:], AF.Exp)
                    ft = foldp.tile([P, CH], F32R, tag="fold")
                    nc.vector.tensor_tensor(
                        ft[:], probs[:, kt, :], probs[:, kt1, :], OP.add
                    )
                    folds.append(ft)
                    if p % 2 == 1:
                        fq = foldp.tile([P, CH], F32R, tag="foldq")
                        nc.vector.tensor_tensor(
                            fq[:], folds[p - 1][:], folds[p][:], OP.add
                        )
                        quads.append(fq)
                    if p % 4 == 3:
                        fo = foldp.tile([P, CH], F32R, tag="foldo")
                        nc.vector.tensor_tensor(
                            fo[:], quads[-2][:], quads[-1][:], OP.add
                        )
                        octs.append(fo)
                    if p >= 1:
                        nc.tensor.matmul(
                            pv[:], vlatq[:, kt - 2, m * P:(m + 1) * P],
                            probs[:, kt - 2, :], start=(p == 1), stop=False,
                        )
                        nc.tensor.matmul(
                            pv[:], vlatq[:, kt - 1, m * P:(m + 1) * P],
                            probs[:, kt - 1, :], start=False, stop=False,
                        )
                    if p == 5:
                        nc.tensor.matmul(
                            pd[:], ones_t[:], octs[0][:],
                            start=True, stop=False,
                        )
                nc.tensor.matmul(
                    pv[:], vlatq[:, NKT - 2, m * P:(m + 1) * P],
                    probs[:, NKT - 2, :], start=False, stop=False,
                )
                nc.tensor.matmul(
                    pv[:], vlatq[:, NKT - 1, m * P:(m + 1) * P],
                    probs[:, NKT - 1, :], start=False, stop=True,
                )
                nc.tensor.matmul(
                    pd[:], ones_t[:], octs[1][:], start=False, stop=True,
                )
                recip = hwork.tile([P, CH], F32, tag="recip")
                nc.vector.reciprocal_approx_fast(recip[:], pd[:])
                nc.vector.tensor_tensor(avT[:, h, :], pv[:], recip[:], OP.mult)

        # ================== phase 3: o_proj in quarter passes ==================
        with ExitStack() as p3:
            outp = p3.enter_context(tc.tile_pool(name="outp", bufs=4))
            ps3 = p3.enter_context(tc.tile_pool(name="ps3", bufs=1, space="PSUM"))

            pre = wo_pre
            for quarter in range(4):
                po = ps3.tile([P, NQT, 512], F32, tag="po", bufs=2)
                for kt in range(H):
                    if kt < len(pre):
                        wo = pre[kt]
                    else:
                        wo = wop.tile([P, 512], BF16, tag="wo")
                        nc.sync.dma_start(
                            wo[:],
                            wo_t[kt * P:(kt + 1) * P,
                                 quarter * 512:(quarter + 1) * 512],
                        )
                    for qc in range(NQT):
                        nc.tensor.matmul(
                            po[:, qc, :],
                            avT[:, kt, qc * P:(qc + 1) * P],
                            wo[:],
                            start=(kt == 0), stop=(kt == H - 1),
                        )
                # prefetch the next quarter's first chunks ahead of the
                # eviction/output DMAs so its matmuls start immediately
                pre = []
                if quarter < 3:
                    for i in range(2):
                        wo = wop.tile([P, 512], BF16, tag="wo")
                        nc.sync.dma_start(
                            wo[:],
                            wo_t[i * P:(i + 1) * P,
                                 (quarter + 1) * 512:(quarter + 2) * 512],
                        )
                        pre.append(wo)
                for qc in range(NQT):
                    ot = outp.tile([P, 512], F32, tag="ot")
                    if qc % 2 == 0:
                        nc.vector.tensor_copy(ot[:], po[:, qc, :])
                    else:
                        nc.scalar.copy(ot[:], po[:, qc, :])
                    nc.sync.dma_start(
                        out_c[
                            qc * P:(qc + 1) * P,
                            quarter * 512:(quarter + 1) * 512,
                        ],
                        ot[:],
                    )

    nc.compile()
    return nc


_NC_CACHE = None


def _get_nc():
    global _NC_CACHE
    if _NC_CACHE is None:
        _NC_CACHE = build_nc()
    return _NC_CACHE


def _prep_in_maps(inputs):
    hidden = np.asarray(inputs["hidden_states"], dtype=np.float32)
    w_qa = np.asarray(inputs["w_qa"], dtype=np.float32)
    ln_qa_g = np.asarray(inputs["ln_qa_g"], dtype=np.float32)
    ln_qa_b = np.asarray(inputs["ln_qa_b"], dtype=np.float32)
    w_qb = np.asarray(inputs["w_qb"], dtype=np.float32)
    w_qrope = np.asarray(inputs["w_qrope"], dtype=np.float32)
    w_kva = np.asarray(inputs["w_kva"], dtype=np.float32)
    ln_kva_g = np.asarray(inputs["ln_kva_g"], dtype=np.float32)
    ln_kva_b = np.asarray(inputs["ln_kva_b"], dtype=np.float32)
    w_kvb = np.asarray(inputs["w_kvb"], dtype=np.float32)
    w_o = np.asarray(inputs["w_o"], dtype=np.float32)
    pos = np.asarray(inputs["position_ids"]).astype(np.int64)

    bf = bfloat16
    hidden_b = hidden.astype(bf)
    hst_all = [
        hidden_b[b].T.reshape(NDT, P, NKT // 2, 2, P).transpose(2, 0, 3, 1, 4)
        for b in range(B)
    ]
    wqa_t = np.ascontiguousarray(w_qa.T.astype(bf))
    # LN gamma folded into q up-projections; beta becomes an output bias:
    # q_nope = (ln0*g + b) @ w_qb.T = ln0 @ (w_qb*g).T + w_qb @ b
    wqb_g = w_qb * ln_qa_g[None, :]
    bqn = (w_qb @ ln_qa_b).astype(np.float32)
    wqb_t = np.ascontiguousarray(wqb_g.T.astype(bf))
    wqr_s = SCALE * w_qrope
    bqr_full = (wqr_s @ ln_qa_b).astype(np.float32)
    wqr_g = (wqr_s * ln_qa_g[None, :]).T
    wqr_t = np.ascontiguousarray(
        wqr_g.reshape(LAT, H, 2, DR // 2).transpose(0, 2, 1, 3)
        .reshape(LAT, H * DR).astype(bf)
    )
    bqr_perm = np.ascontiguousarray(
        bqr_full.reshape(H, 2, DR // 2).transpose(1, 0, 2).reshape(H * DR)
    )
    wkva_t = np.ascontiguousarray(w_kva.T.astype(bf))
    kup = (SCALE * w_kvb[: H * DN]).reshape(H, DN, R)
    bkn = (kup @ ln_kva_b).reshape(H * DN).astype(np.float32)
    kup_g = kup * ln_kva_g[None, None, :]
    kup_t = np.ascontiguousarray(
        kup_g.transpose(2, 0, 1).reshape(R, H * DN).astype(bf)
    )
    vup = w_kvb[H * DN:].reshape(H, DV, R)
    bvv = (vup @ ln_kva_b).reshape(H * DV).astype(np.float32)
    vup_g = vup * ln_kva_g[None, None, :]
    vup_t = np.ascontiguousarray(
        vup_g.transpose(2, 0, 1).reshape(R, H * DV).astype(bf)
    )
    wo_t = np.ascontiguousarray(w_o.T.astype(bf))
    ones_in = np.ones((P, P), dtype=np.float32)

    inv_freq = 1.0 / (10000.0 ** (np.arange(0, DR, 2, dtype=np.float64) / DR))
    ang = pos[:, None].astype(np.float64) * inv_freq[None, :]
    cosf = np.ascontiguousarray(np.cos(ang).astype(np.float32))
    sinf = np.ascontiguousarray(np.sin(ang).astype(np.float32))

    in_maps = []
    for c in range(N_CORES):
        b, ch = divmod(c, NQT)
        qs = ch * CH
        cq = np.ascontiguousarray(np.tile(cosf[qs:qs + CH, :].T, (NQT, 1)))
        sq = np.ascontiguousarray(np.tile(sinf[qs:qs + CH, :].T, (NQT, 1)))
        # keys are reordered so this core's own q-chunk pairs come first
        # (attention is permutation-invariant over keys); the rope tables
        # below follow the same order
        myp = [2 * ch, 2 * ch + 1]
        rest = [p for p in range(NKT // 2) if p not in myp]
        key_perm = np.concatenate(
            [np.arange(256 * p, 256 * (p + 1)) for p in myp + rest]
        )
        in_maps.append({
            "hst4": np.ascontiguousarray(hst_all[b][rest]),
            "hsqt": np.ascontiguousarray(hst_all[b][myp]),
            "wqa_t": wqa_t,
            "wqb_t": wqb_t,
            "wqr_t": wqr_t,
            "wkva_t": wkva_t,
            "kup_t": kup_t,
            "vup_t": vup_t,
            "wo_t": wo_t,
            "bqn_v": bqn,
            "bqr_v": bqr_perm,
            "bkn_v": bkn,
            "bvv_v": bvv,
            "ones_in": ones_in,
            "ck_tab": np.ascontiguousarray(cosf[key_perm]),
            "sk_tab": np.ascontiguousarray(sinf[key_perm]),
            "cq_tab": cq,
            "sq_tab": sq,
        })
    return in_maps


def kernel(**inputs) -> np.ndarray:
    nc = _get_nc()
    in_maps = _prep_in_maps(inputs)
    res = run_bass_kernel_spmd(nc, in_maps, core_ids=list(range(N_CORES)))
    out = np.empty((B, S, D), dtype=np.float32)
    for c in range(N_CORES):
        b, ch = divmod(c, NQT)
        out[b, ch * CH:(ch + 1) * CH, :] = res.results[c]["out_c"]
    return out


# revision 42
# speedup vs baseline: 1.0236x; 1.0236x over previous
"""MLA (multi-head latent attention) prefill kernel for 8 Trainium2 NeuronCores.

Sharding: pure data-parallel over (batch, query-chunk). Core c handles batch
c//4 and query rows [512*(c%4), 512*(c%4+1)). The KV path (latent + k_rope,
full 2048 keys) is recomputed per core from the batch's hidden states
(~9% FLOP overhead) so there are ZERO collectives; every core writes a
disjoint [512, 2048] slice of the output.

Structure (final):
 - De-absorbed prefill math: k_nope[h] = kv_lat @ k_up[h]^T and
   v_lat[h] = kv_lat @ v_up[h]^T materialized per head(-group): scores
   contract 128+64 channels (not 512+64), attn rank 128 (not 512).
 - All matmuls bf16 (fp32 PSUM) for the PE fast-weight-load path.
 - LN gammas folded into consuming weights, betas into eviction biases;
   LN itself is one fused (x-mu)*rstd vector op.
 - Phase 1 is software-pipelined in segments: all projection matmuls run
   back-to-back while LayerNorms trail on vector/scalar; transposes are
   batched (5 per PSUM bank, single strided eviction) and interleaved
   with the group-0 attention prologue so the PE never waits on LN.
 - RoPE score matmuls (K=64) packed pairwise onto disjoint PE row groups.
 - Softmax denominator: probs folded pair->quad->oct on vector, two
   ones-matmuls per head; fast approximate reciprocal.
 - attn_v^T stays in SBUF as the o_proj lhsT; o_proj runs as 4 quarter
   passes with double-buffered PSUM so evictions overlap matmuls.
"""

import math
from contextlib import ExitStack

import numpy as np
from ml_dtypes import bfloat16

import concourse.bass as bass
import concourse.tile as tile
from concourse import bacc, mybir
from concourse.bass_utils import run_bass_kernel_spmd
from concourse.masks import make_identity

F32 = mybir.dt.float32
F32R = mybir.dt.float32r
BF16 = mybir.dt.bfloat16
AF = mybir.ActivationFunctionType
OP = mybir.AluOpType

B, S, D = 2, 2048, 2048
H = 16
LAT = 1536
R = 512
DN, DR, DV = 128, 64, 128
EPS = 1e-5
SCALE = 1.0 / math.sqrt(DN + DR)

P = 128
CH = 512
NQT = CH // P
NKT = S // P
NDT = D // P
NLT = LAT // P

N_CORES = 8


def _bcast_rows(t, n, length):
    return bass.AP(tensor=t, offset=0, ap=[[0, n], [1, length]])


def build_nc():
    nc = bacc.Bacc(None, target_bir_lowering=False)

    hst4 = nc.dram_tensor("hst4", [NKT // 2 - 2, NDT, 2, P, P], BF16,
                          kind="ExternalInput")
    hsqt = nc.dram_tensor("hsqt", [2, NDT, 2, P, P], BF16, kind="ExternalInput")
    wqa_t = nc.dram_tensor("wqa_t", [D, LAT], BF16, kind="ExternalInput")
    wqb_t = nc.dram_tensor("wqb_t", [LAT, H * DN], BF16, kind="ExternalInput")
    wqr_t = nc.dram_tensor("wqr_t", [LAT, H * DR], BF16, kind="ExternalInput")
    wkva_t = nc.dram_tensor("wkva_t", [D, R + DR], BF16, kind="ExternalInput")
    kup_t = nc.dram_tensor("kup_t", [R, H * DN], BF16, kind="ExternalInput")
    vup_t = nc.dram_tensor("vup_t", [R, H * DV], BF16, kind="ExternalInput")
    wo_t = nc.dram_tensor("wo_t", [H * DV, D], BF16, kind="ExternalInput")
    bqn_v = nc.dram_tensor("bqn_v", [H * DN], F32, kind="ExternalInput")
    bqr_v = nc.dram_tensor("bqr_v", [H * DR], F32, kind="ExternalInput")
    bkn_v = nc.dram_tensor("bkn_v", [H * DN], F32, kind="ExternalInput")
    bvv_v = nc.dram_tensor("bvv_v", [H * DV], F32, kind="ExternalInput")
    ones_in = nc.dram_tensor("ones_in", [P, P], F32R, kind="ExternalInput")
    ck_tab = nc.dram_tensor("ck_tab", [S, DR // 2], F32, kind="ExternalInput")
    sk_tab = nc.dram_tensor("sk_tab", [S, DR // 2], F32, kind="ExternalInput")
    cq_tab = nc.dram_tensor("cq_tab", [P, CH], F32, kind="ExternalInput")
    sq_tab = nc.dram_tensor("sq_tab", [P, CH], F32, kind="ExternalInput")
    out_c = nc.dram_tensor("out_c", [CH, D], F32, kind="ExternalOutput")

    with tile.TileContext(nc) as tc, ExitStack() as octx:
        res = octx.enter_context(tc.tile_pool(name="res", bufs=1))
        # k_full^T: 4x128 latent planes; plane 4 = rope channels duplicated
        # to partitions 64:128 for row-group packing of the rope matmuls.
        kfull = res.tile([P, 5, S], BF16)
        qlat_t = res.tile([P, NLT, CH], BF16)
        kupT = res.tile([P, 4, H * DN], BF16)
        vupT = res.tile([P, 4, H * DV], BF16)

        consts = octx.enter_context(tc.tile_pool(name="consts", bufs=1))
        ident = consts.tile([P, P], BF16)
        make_identity(nc, ident)
        ones_t = consts.tile([P, P], F32R)
        eps_t = consts.tile([P, 1], F32)
        nc.vector.memset(eps_t, EPS)
        cq_t = consts.tile([P, CH], F32)
        sq_t = consts.tile([P, CH], F32)
        bqn_t = consts.tile([P, H], F32)
        bqr_t = consts.tile([P, 8], F32)
        bkn_t = consts.tile([P, H], F32)
        bvv_bc = consts.tile([P, H * DV], F32)

        wop = octx.enter_context(tc.tile_pool(name="wop", bufs=6))

        # group-0 attention prologue tiles (built during phase 1 segment C)
        qwork = octx.enter_context(tc.tile_pool(name="qwork", bufs=1))
        gwork = octx.enter_context(tc.tile_pool(name="gwork", bufs=1))

        with ExitStack() as p1all:
            wkvp = p1all.enter_context(tc.tile_pool(name="wkvp", bufs=1))
            hsqp = p1all.enter_context(tc.tile_pool(name="hsqp", bufs=1))
            lnsp = p1all.enter_context(tc.tile_pool(name="lnsp", bufs=1))
            qln_all = lnsp.tile([P, NQT, LAT], BF16)
            lnf_all = lnsp.tile([P, NKT, R], BF16)
            kro_all = lnsp.tile([P, NKT, P], BF16)

            # ================= segment A: q-mix matmuls =================
            with ExitStack() as pA:
                wqap = pA.enter_context(tc.tile_pool(name="wqap", bufs=4))
                mixp = pA.enter_context(tc.tile_pool(name="mixp", bufs=1))
                lnp = pA.enter_context(tc.tile_pool(name="lnp", bufs=2))
                psA = pA.enter_context(tc.tile_pool(name="psA", bufs=1, space="PSUM"))

                qmix_all = mixp.tile([P, NQT, LAT], BF16)
                wkv_all = wkvp.tile([P, NDT, R + DR], BF16)
                ck_t = wkvp.tile([P, NKT, DR // 2], F32)
                sk_t = wkvp.tile([P, NKT, DR // 2], F32)

                # heavy phase-1b/2 weight DMAs are spread through the j/dt
                # loop so they never serialize ahead of the q-mix operands
                def _stream_weights(j, dt):
                    if j == 0 and dt % 4 == 2:
                        a = dt // 4
                        nc.sync.dma_start(
                            wkv_all[:, 4 * a:4 * (a + 1), :],
                            wkva_t[512 * a:512 * (a + 1), :].rearrange(
                                "(t p) c -> p t c", p=P
                            ),
                        )
                    elif j == 0 and dt == 13:
                        nc.sync.dma_start(
                            ck_t[:], ck_tab.ap().rearrange("(t p) j -> p t j", p=P)
                        )
                        nc.sync.dma_start(
                            sk_t[:], sk_tab.ap().rearrange("(t p) j -> p t j", p=P)
                        )
                    elif j == 1 and dt % 4 == 2:
                        rc = dt // 4
                        nc.sync.dma_start(
                            kupT[:, rc, :], kup_t[rc * P:(rc + 1) * P, :]
                        )
                    elif j == 2 and dt % 4 == 2:
                        rc = dt // 4
                        nc.sync.dma_start(
                            vupT[:, rc, :], vup_t[rc * P:(rc + 1) * P, :]
                        )
                    elif j == 1 and dt == 13:
                        nc.sync.dma_start(cq_t[:], cq_tab[:, :])
                        nc.sync.dma_start(sq_t[:], sq_tab[:, :])
                    elif j == 2 and dt == 15:
                        nc.sync.dma_start(ones_t[:], ones_in[:, :])
                        nc.sync.dma_start(
                            bqn_t[:], bqn_v.ap().rearrange("(h p) -> p h", p=P)
                        )
                        nc.sync.dma_start(
                            bqr_t[:], bqr_v.ap().rearrange("(a p) -> p a", p=P)
                        )
                        nc.sync.dma_start(
                            bkn_t[:], bkn_v.ap().rearrange("(h p) -> p h", p=P)
                        )
                        nc.sync.dma_start(bvv_bc[:], _bcast_rows(bvv_v, P, H * DV))

                hsq_all = hsqp.tile([P, 2, NDT, 2, P], BF16)
                for pair in range(2):
                    nc.sync.dma_start(
                        hsq_all[:, pair, 0:1, :, :],
                        hsqt[pair, 0:1].rearrange("d k p c -> p d k c"),
                    )
                for pair in range(2):
                    nc.sync.dma_start(
                        hsq_all[:, pair, 1:4, :, :],
                        hsqt[pair, 1:4].rearrange("d k p c -> p d k c"),
                    )
                for a in range(1, 4):
                    for pair in range(2):
                        nc.sync.dma_start(
                            hsq_all[:, pair, 4 * a:4 * (a + 1), :, :],
                            hsqt[pair, 4 * a:4 * (a + 1)].rearrange(
                                "d k p c -> p d k c"
                            ),
                        )
                for j in range(3):
                    pqj = psA.tile([P, NQT, 512], F32, tag="pq", bufs=2)
                    for a in range(4):
                        wqa_c = wqap.tile([P, 4, 512], BF16, tag="wqa")
                        if j == 0 and a == 0:
                            nc.sync.dma_start(
                                wqa_c[:, 0:1, :],
                                wqa_t[0:P, 0:512].rearrange(
                                    "(t p) c -> p t c", p=P
                                ),
                            )
                            nc.sync.dma_start(
                                wqa_c[:, 1:4, :],
                                wqa_t[P:512, 0:512].rearrange(
                                    "(t p) c -> p t c", p=P
                                ),
                            )
                        else:
                            nc.sync.dma_start(
                                wqa_c[:],
                                wqa_t[a * 512:(a + 1) * 512,
                                      j * 512:(j + 1) * 512].rearrange(
                                    "(t p) c -> p t c", p=P
                                ),
                            )
                        for i in range(4):
                            dt = 4 * a + i
                            for qt in range(NQT):
                                nc.tensor.matmul(
                                    pqj[:, qt, :],
                                    hsq_all[:, qt // 2, dt, qt % 2, :],
                                    wqa_c[:, i, :],
                                    start=(dt == 0), stop=(dt == NDT - 1),
                                )
                            _stream_weights(j, dt)
                    for qt in range(NQT):
                        nc.vector.tensor_copy(
                            qmix_all[:, qt, j * 512:(j + 1) * 512], pqj[:, qt, :]
                        )

                # q LayerNorms (vector trails; consumed by segment C)
                for qt in range(NQT):
                    statsq = lnp.tile([P, 3, 6], F32, tag="statsq")
                    for j in range(3):
                        nc.vector.bn_stats(
                            statsq[:, j, :], qmix_all[:, qt, j * 512:(j + 1) * 512]
                        )
                    mvq = lnp.tile([P, 2], F32, tag="mv")
                    nc.vector.bn_aggr(mvq[:], statsq[:])
                    rstdq = lnp.tile([P, 1], F32, tag="rstd")
                    nc.scalar.activation(
                        rstdq[:], mvq[:, 1:2], AF.Sqrt, bias=eps_t[:]
                    )
                    nc.vector.reciprocal(rstdq[:], rstdq[:])
                    nc.vector.tensor_scalar(
                        qln_all[:, qt, :], qmix_all[:, qt, :], mvq[:, 0:1],
                        rstdq[:], op0=OP.subtract, op1=OP.mult,
                    )

            # ============= segment B: kv-mix matmuls + trailing LN =============
            with ExitStack() as pB:
                hsl = pB.enter_context(tc.tile_pool(name="hsl", bufs=6))
                mixp = pB.enter_context(tc.tile_pool(name="mixp", bufs=3))
                lnp = pB.enter_context(tc.tile_pool(name="lnp", bufs=2))
                psB = pB.enter_context(tc.tile_pool(name="psB", bufs=1, space="PSUM"))

                hk_pre = []
                for a in range(4):
                    hk8 = hsl.tile([P, 4, 2, P], BF16, tag="hs", name="hkpre")
                    nc.sync.dma_start(
                        hk8[:],
                        hst4[0, 4 * a:4 * (a + 1)].rearrange(
                            "d k p c -> p d k c"
                        ),
                    )
                    hk_pre.append(hk8)
                for ktp in range(NKT // 2):
                    kt0 = 2 * ktp
                    pm = [
                        psB.tile([P, 2, 512], F32, tag=f"pmix{i}", bufs=2,
                                 name=f"pm{i}")
                        for i in range(2)
                    ]
                    for a in range(4):
                        if ktp < 2:
                            # this kt-pair is the core's own q-chunk: tiles
                            # are already resident from segment A
                            hk8 = hsq_all[:, ktp, 4 * a:4 * (a + 1), :, :]
                        elif ktp == 2:
                            hk8 = hk_pre[a]
                        else:
                            hk8t = hsl.tile([P, 4, 2, P], BF16, tag="hs")
                            nc.sync.dma_start(
                                hk8t[:],
                                hst4[ktp - 2, 4 * a:4 * (a + 1)].rearrange(
                                    "d k p c -> p d k c"
                                ),
                            )
                            hk8 = hk8t
                        for di in range(4):
                            dt = 4 * a + di
                            st = (dt == 0)
                            sp = (dt == NDT - 1)
                            for i in range(2):
                                nc.tensor.matmul(
                                    pm[i][:, 0, 0:288], hk8[:, di, i, :],
                                    wkv_all[:, dt, 0:288], start=st, stop=sp,
                                )
                                nc.tensor.matmul(
                                    pm[i][:, 1, 0:288], hk8[:, di, i, :],
                                    wkv_all[:, dt, 288:576], start=st, stop=sp,
                                )
                    for i in range(2):
                        kt = kt0 + i
                        kvmix = mixp.tile([P, R + DR], F32, tag="kvmix")
                        nc.scalar.copy(kvmix[:, 0:288], pm[i][:, 0, 0:288])
                        nc.scalar.copy(kvmix[:, 288:576], pm[i][:, 1, 0:288])

                        stats = lnp.tile([P, 6], F32, tag="stats")
                        nc.vector.bn_stats(stats[:], kvmix[:, 0:R])
                        mv = lnp.tile([P, 2], F32, tag="mv")
                        nc.vector.bn_aggr(mv[:], stats[:])
                        rstd = lnp.tile([P, 1], F32, tag="rstd")
                        nc.scalar.activation(
                            rstd[:], mv[:, 1:2], AF.Sqrt, bias=eps_t[:]
                        )
                        nc.vector.reciprocal(rstd[:], rstd[:])
                        nc.vector.tensor_scalar(
                            lnf_all[:, kt, :], kvmix[:, 0:R], mv[:, 0:1],
                            rstd[:], op0=OP.subtract, op1=OP.mult,
                        )

                        # RoPE, rotated pairs duplicated to cols 64:128
                        t1 = lnp.tile([P, DR // 2], F32, tag="t1")
                        t2 = lnp.tile([P, DR // 2], F32, tag="t2")
                        x1 = kvmix[:, R:R + 32]
                        x2 = kvmix[:, R + 32:R + 64]
                        kro = kro_all[:, kt, :]
                        nc.vector.tensor_tensor(t2[:], x1, ck_t[:, kt, :], OP.mult)
                        nc.vector.tensor_tensor(t1[:], x2, sk_t[:, kt, :], OP.mult)
                        nc.vector.tensor_tensor(kro[:, 0:32], t2[:], t1[:], OP.subtract)
                        nc.vector.tensor_tensor(kro[:, 64:96], t2[:], t1[:], OP.subtract)
                        nc.vector.tensor_tensor(t2[:], x1, sk_t[:, kt, :], OP.mult)
                        nc.vector.tensor_tensor(t1[:], x2, ck_t[:, kt, :], OP.mult)
                        nc.vector.tensor_tensor(kro[:, 32:64], t2[:], t1[:], OP.add)
                        nc.vector.tensor_tensor(kro[:, 96:128], t2[:], t1[:], OP.add)

            # ==== segment C: batched transposes + group-0 attention prologue ====
            with ExitStack() as pC:
                psC = pC.enter_context(tc.tile_pool(name="psC", bufs=1, space="PSUM"))
                wqs0 = pC.enter_context(tc.tile_pool(name="wqs0", bufs=2))

                # q latent transposes: 6 per PSUM bank, one strided evict each
                ev = 0
                for qt in range(NQT):
                    for half in range(2):
                        ptq = psC.tile([P, 6, P], BF16, tag="ptr", bufs=2)
                        for i in range(6):
                            lt = half * 6 + i
                            nc.tensor.transpose(
                                ptq[:, i, :],
                                qln_all[:, qt, lt * P:(lt + 1) * P], ident[:],
                            )
                        dst = qlat_t[:, half * 6:(half + 1) * 6,
                                     qt * P:(qt + 1) * P]
                        if ev % 2 == 0:
                            nc.vector.tensor_copy(dst, ptq[:])
                        else:
                            nc.scalar.copy(dst, ptq[:])
                        ev += 1

                # group-0 q_rope projection + rotation
                qraw = qwork.tile([P, 2, CH], F32, tag="qraw")
                for half in range(2):
                    wrc = wqs0.tile([P, NLT, P], BF16, tag="wq")
                    nc.sync.dma_start(
                        wrc[:],
                        wqr_t[:, half * 512:half * 512 + P].rearrange(
                            "(t p) c -> p t c", p=P
                        ),
                    )
                    pr = psC.tile([P, 512], F32, tag="proj", bufs=2)
                    for lt in range(NLT):
                        nc.tensor.matmul(
                            pr[:], wrc[:, lt, :], qlat_t[:, lt, :],
                            start=(lt == 0), stop=(lt == NLT - 1),
                        )
                    nc.scalar.add(
                        qraw[:, half, :], pr[:], bqr_t[:, half * 4:half * 4 + 1]
                    )
                qro0 = qwork.tile([P, 2, CH], BF16, tag="qro")
                tm = qwork.tile([P, CH], F32, tag="tm")
                tn = qwork.tile([P, CH], F32, tag="tn")
                x1, x2 = qraw[:, 0, :], qraw[:, 1, :]
                nc.vector.tensor_tensor(tm[:], x2, sq_t[:], OP.mult)
                nc.vector.tensor_tensor(tn[:], x1, cq_t[:], OP.mult)
                nc.vector.tensor_tensor(qro0[:, 0, :], tn[:], tm[:], OP.subtract)
                nc.vector.tensor_tensor(tm[:], x2, cq_t[:], OP.mult)
                nc.vector.tensor_tensor(tn[:], x1, sq_t[:], OP.mult)
                nc.vector.tensor_tensor(qro0[:, 1, :], tn[:], tm[:], OP.add)

                # kv transposes (5 per bank, single evict) + group-0 v_lat
                vlatq0 = gwork.tile([P, NKT, 512], BF16, tag="vlatq", bufs=1)
                for kt in range(NKT):
                    pt = psC.tile([P, 5, P], BF16, tag="ptr", bufs=2)
                    for j in range(4):
                        nc.tensor.transpose(
                            pt[:, j, :],
                            lnf_all[:, kt, j * P:(j + 1) * P], ident[:],
                        )
                    nc.tensor.transpose(pt[:, 4, :], kro_all[:, kt, :], ident[:])
                    dst = kfull[:, 0:5, kt * P:(kt + 1) * P]
                    if kt % 2 == 0:
                        nc.vector.tensor_copy(dst, pt[:])
                    else:
                        nc.scalar.copy(dst, pt[:])

                    pv1 = psC.tile([P, 512], F32, tag="proj", bufs=2)
                    for rc in range(4):
                        nc.tensor.matmul(
                            pv1[:], kfull[:, rc, kt * P:(kt + 1) * P],
                            vupT[:, rc, 0:512],
                            start=(rc == 0), stop=(rc == 3),
                        )
                    nc.vector.tensor_tensor(
                        vlatq0[:, kt, :], pv1[:], bvv_bc[:, 0:512], OP.add
                    )

        # ====================== phase 2: attention head loop ======================
        attp = octx.enter_context(tc.tile_pool(name="attp", bufs=1))
        avT = attp.tile([P, H, CH], BF16)

        wo_pre = []
        with ExitStack() as p2:
            wqs = p2.enter_context(tc.tile_pool(name="wqs", bufs=2))
            hwork = p2.enter_context(tc.tile_pool(name="hwork", bufs=2))
            probs_p = p2.enter_context(tc.tile_pool(name="probs_p", bufs=2))
            foldp = p2.enter_context(tc.tile_pool(name="foldp", bufs=3))
            ps2 = p2.enter_context(tc.tile_pool(name="ps2", bufs=1, space="PSUM"))

            qro, vlatq = qro0, vlatq0
            for h in range(H):
                g, m = divmod(h, 4)
                if m == 0 and g > 0:
                    qraw = qwork.tile([P, 2, CH], F32, tag="qraw")
                    for half in range(2):
                        wrc = wqs.tile([P, NLT, P], BF16, tag="wq")
                        col0 = half * 512 + g * P
                        nc.sync.dma_start(
                            wrc[:],
                            wqr_t[:, col0:col0 + P].rearrange(
                                "(t p) c -> p t c", p=P
                            ),
                        )
                        pr = ps2.tile([P, 512], F32, tag="proj", bufs=2)
                        for lt in range(NLT):
                            nc.tensor.matmul(
                                pr[:], wrc[:, lt, :], qlat_t[:, lt, :],
                                start=(lt == 0), stop=(lt == NLT - 1),
                            )
                        nc.scalar.add(
                            qraw[:, half, :], pr[:],
                            bqr_t[:, half * 4 + g:half * 4 + g + 1],
                        )
                    qro = qwork.tile([P, 2, CH], BF16, tag="qro")
                    tm = qwork.tile([P, CH], F32, tag="tm")
                    tn = qwork.tile([P, CH], F32, tag="tn")
                    x1, x2 = qraw[:, 0, :], qraw[:, 1, :]
                    nc.vector.tensor_tensor(tm[:], x2, sq_t[:], OP.mult)
                    nc.vector.tensor_tensor(tn[:], x1, cq_t[:], OP.mult)
                    nc.vector.tensor_tensor(qro[:, 0, :], tn[:], tm[:], OP.subtract)
                    nc.vector.tensor_tensor(tm[:], x2, cq_t[:], OP.mult)
                    nc.vector.tensor_tensor(tn[:], x1, sq_t[:], OP.mult)
                    nc.vector.tensor_tensor(qro[:, 1, :], tn[:], tm[:], OP.add)

                    vlatq = gwork.tile([P, NKT, 512], BF16, tag="vlatq", bufs=1)
                    for kt in range(NKT):
                        pv1 = ps2.tile([P, 512], F32, tag="proj", bufs=2)
                        for rc in range(4):
                            nc.tensor.matmul(
                                pv1[:], kfull[:, rc, kt * P:(kt + 1) * P],
                                vupT[:, rc, g * 512:(g + 1) * 512],
                                start=(rc == 0), stop=(rc == 3),
                            )
                        nc.vector.tensor_tensor(
                            vlatq[:, kt, :], pv1[:],
                            bvv_bc[:, g * 512:(g + 1) * 512], OP.add,
                        )

                if h == H - 1:
                    for i in range(3):
                        wo = wop.tile([P, 512], BF16, tag="wo")
                        nc.sync.dma_start(wo[:], wo_t[i * P:(i + 1) * P, 0:512])
                        wo_pre.append(wo)

                wb = wqs.tile([P, NLT, P], BF16, tag="wq")
                nc.sync.dma_start(
                    wb[:],
                    wqb_t[:, h * P:(h + 1) * P].rearrange("(t p) c -> p t c", p=P),
                )
                pn = ps2.tile([P, 512], F32, tag="proj", bufs=2)
                for lt in range(NLT):
                    nc.tensor.matmul(
                        pn[:], wb[:, lt, :], qlat_t[:, lt, :],
                        start=(lt == 0), stop=(lt == NLT - 1),
                    )
                qnope = hwork.tile([P, CH], BF16, tag="qnope")
                nc.scalar.add(qnope[:], pn[:], bqn_t[:, h:h + 1])

                knopeT = hwork.tile([P, S], BF16, tag="knopeT")
                for kc in range(4):
                    pk = ps2.tile([P, 512], F32, tag="proj", bufs=2)
                    for rc in range(4):
                        nc.tensor.matmul(
                            pk[:], kupT[:, rc, h * P:(h + 1) * P],
                            kfull[:, rc, kc * 512:(kc + 1) * 512],
                            start=(rc == 0), stop=(rc == 3),
                        )
                    nc.scalar.add(
                        knopeT[:, kc * 512:(kc + 1) * 512], pk[:],
                        bkn_t[:, h:h + 1],
                    )

                qropeT = hwork.tile([P, CH], BF16, tag="qropeT")
                nc.sync.dma_start(qropeT[0:32, :], qro[m * 32:(m + 1) * 32, 0, :])
                nc.sync.dma_start(qropeT[32:64, :], qro[m * 32:(m + 1) * 32, 1, :])
                nc.sync.dma_start(qropeT[64:96, :], qro[m * 32:(m + 1) * 32, 0, :])
                nc.sync.dma_start(qropeT[96:128, :], qro[m * 32:(m + 1) * 32, 1, :])

                probs = probs_p.tile([P, NKT, CH], BF16, tag="probs")
                folds = []
                quads = []
                octs = []
                pv = ps2.tile([P, 512], F32, tag="attn", bufs=1)
                pd = ps2.tile([P, 512], F32, tag="den", bufs=1)
                for p in range(NKT // 2):
                    kt, kt1 = 2 * p, 2 * p + 1
                    sc = ps2.tile([P, 2, 512], F32, tag="scores", bufs=2)
                    nc.tensor.matmul(
                        sc[:, 0, :], knopeT[:, kt * P:(kt + 1) * P], qnope[:],
                        start=True, stop=False,
                    )
                    nc.tensor.matmul(
                        sc[:, 1, :], knopeT[:, kt1 * P:(kt1 + 1) * P], qnope[:],
                        start=True, stop=False,
                    )
                    nc.tensor.matmul(
                        sc[:, 0, :], kfull[0:DR, 4, kt * P:(kt + 1) * P],
                        qropeT[0:DR, :], start=False, stop=True,
                    )
                    nc.tensor.matmul(
                        sc[:, 1, :], kfull[DR:P, 4, kt1 * P:(kt1 + 1) * P],
                        qropeT[DR:P, :], start=False, stop=True,
                        tile_position=(DR, 0),
                    )
                    nc.scalar.activation(probs[:, kt:kt + 2, :], sc[:], AF.Exp)
                    ft = foldp.tile([P, CH], F32R, tag="fold")
                    nc.vector.tensor_tensor(
                        ft[:], probs[:, kt, :], probs[:, kt1, :], OP.add
                    )
                    folds.append(ft)
                    if p % 2 == 1:
                        fq = foldp.tile([P, CH], F32R, tag="foldq")
                        nc.vector.tensor_tensor(
                            fq[:], folds[p - 1][:], folds[p][:], OP.add
                        )
                        quads.append(fq)
                    if p % 4 == 3:
                        fo = foldp.tile([P, CH], F32R, tag="foldo")
                        nc.vector.tensor_tensor(
                            fo[:], quads[-2][:], quads[-1][:], OP.add
                        )
                        octs.append(fo)
                    if p >= 1:
                        nc.tensor.matmul(
                            pv[:], vlatq[:, kt - 2, m * P:(m + 1) * P],
                            probs[:, kt - 2, :], start=(p == 1), stop=False,
                        )
                        nc.tensor.matmul(
                            pv[:], vlatq[:, kt - 1, m * P:(m + 1) * P],
                            probs[:, kt - 1, :], start=False, stop=False,
                        )
                    if p == 5:
                        nc.tensor.matmul(
                            pd[:], ones_t[:], octs[0][:],
                            start=True, stop=False,
                        )
                nc.tensor.matmul(
                    pv[:], vlatq[:, NKT - 2, m * P:(m + 1) * P],
                    probs[:, NKT - 2, :], start=False, stop=False,
                )
                nc.tensor.matmul(
                    pv[:], vlatq[:, NKT - 1, m * P:(m + 1) * P],
                    probs[:, NKT - 1, :], start=False, stop=True,
                )
                nc.tensor.matmul(
                    pd[:], ones_t[:], octs[1][:], start=False, stop=True,
                )
                recip = hwork.tile([P, CH], F32, tag="recip")
                nc.vector.reciprocal_approx_fast(recip[:], pd[:])
                nc.vector.tensor_tensor(avT[:, h, :], pv[:], recip[:], OP.mult)

        # ================== phase 3: o_proj in quarter passes ==================
        with ExitStack() as p3:
            outp = p3.enter_context(tc.tile_pool(name="outp", bufs=4))
            ps3 = p3.enter_context(tc.tile_pool(name="ps3", bufs=1, space="PSUM"))

            pre = wo_pre
            for quarter in range(4):
                po = ps3.tile([P, NQT, 512], F32, tag="po", bufs=2)
                for kt in range(H):
                    if kt < len(pre):
                        wo = pre[kt]
                    else:
                        wo = wop.tile([P, 512], BF16, tag="wo")
                        nc.sync.dma_start(
                            wo[:],
                            wo_t[kt * P:(kt + 1) * P,
                                 quarter * 512:(quarter + 1) * 512],
                        )
                    for qc in range(NQT):
                        nc.tensor.matmul(
                            po[:, qc, :],
                            avT[:, kt, qc * P:(qc + 1) * P],
                            wo[:],
                            start=(kt == 0), stop=(kt == H - 1),
                        )
                # prefetch the next quarter's first chunks ahead of the
                # eviction/output DMAs so its matmuls start immediately
                pre = []
                if quarter < 3:
                    for i in range(2):
                        wo = wop.tile([P, 512], BF16, tag="wo")
                        nc.sync.dma_start(
                            wo[:],
                            wo_t[i * P:(i + 1) * P,
                                 (quarter + 1) * 512:(quarter + 2) * 512],
                        )
                        pre.append(wo)
                for qc in range(NQT):
                    ot = outp.tile([P, 512], F32, tag="ot")
                    if qc % 2 == 0:
                        nc.vector.tensor_copy(ot[:], po[:, qc, :])
                    else:
                        nc.scalar.copy(ot[:], po[:, qc, :])
                    nc.sync.dma_start(
                        out_c[
                            qc * P:(qc + 1) * P,
                            quarter * 512:(quarter + 1) * 512,
                        ],
                        ot[:],
                    )

    nc.compile()
    return nc


_NC_CACHE = None


def _get_nc():
    global _NC_CACHE
    if _NC_CACHE is None:
        _NC_CACHE = build_nc()
    return _NC_CACHE


def _prep_in_maps(inputs):
    hidden = np.asarray(inputs["hidden_states"], dtype=np.float32)
    w_qa = np.asarray(inputs["w_qa"], dtype=np.float32)
    ln_qa_g = np.asarray(inputs["ln_qa_g"], dtype=np.float32)
    ln_qa_b = np.asarray(inputs["ln_qa_b"], dtype=np.float32)
    w_qb = np.asarray(inputs["w_qb"], dtype=np.float32)
    w_qrope = np.asarray(inputs["w_qrope"], dtype=np.float32)
    w_kva = np.asarray(inputs["w_kva"], dtype=np.float32)
    ln_kva_g = np.asarray(inputs["ln_kva_g"], dtype=np.float32)
    ln_kva_b = np.asarray(inputs["ln_kva_b"], dtype=np.float32)
    w_kvb = np.asarray(inputs["w_kvb"], dtype=np.float32)
    w_o = np.asarray(inputs["w_o"], dtype=np.float32)
    pos = np.asarray(inputs["position_ids"]).astype(np.int64)

    bf = bfloat16
    hidden_b = hidden.astype(bf)
    hst_all = [
        hidden_b[b].T.reshape(NDT, P, NKT // 2, 2, P).transpose(2, 0, 3, 1, 4)
        for b in range(B)
    ]
    wqa_t = np.ascontiguousarray(w_qa.T.astype(bf))
    # LN gamma folded into q up-projections; beta becomes an output bias:
    # q_nope = (ln0*g + b) @ w_qb.T = ln0 @ (w_qb*g).T + w_qb @ b
    wqb_g = w_qb * ln_qa_g[None, :]
    bqn = (w_qb @ ln_qa_b).astype(np.float32)
    wqb_t = np.ascontiguousarray(wqb_g.T.astype(bf))
    wqr_s = SCALE * w_qrope
    bqr_full = (wqr_s @ ln_qa_b).astype(np.float32)
    wqr_g = (wqr_s * ln_qa_g[None, :]).T
    wqr_t = np.ascontiguousarray(
        wqr_g.reshape(LAT, H, 2, DR // 2).transpose(0, 2, 1, 3)
        .reshape(LAT, H * DR).astype(bf)
    )
    bqr_perm = np.ascontiguousarray(
        bqr_full.reshape(H, 2, DR // 2).transpose(1, 0, 2).reshape(H * DR)
    )
    wkva_t = np.ascontiguousarray(w_kva.T.astype(bf))
    kup = (SCALE * w_kvb[: H * DN]).reshape(H, DN, R)
    bkn = (kup @ ln_kva_b).reshape(H * DN).astype(np.float32)
    kup_g = kup * ln_kva_g[None, None, :]
    kup_t = np.ascontiguousarray(
        kup_g.transpose(2, 0, 1).reshape(R, H * DN).astype(bf)
    )
    vup = w_kvb[H * DN:].reshape(H, DV, R)
    bvv = (vup @ ln_kva_b).reshape(H * DV).astype(np.float32)
    vup_g = vup * ln_kva_g[None, None, :]
    vup_t = np.ascontiguousarray(
        vup_g.transpose(2, 0, 1).reshape(R, H * DV).astype(bf)
    )
    wo_t = np.ascontiguousarray(w_o.T.astype(bf))
    ones_in = np.ones((P, P), dtype=np.float32)

    inv_freq = 1.0 / (10000.0 ** (np.arange(0, DR, 2, dtype=np.float64) / DR))
    ang = pos[:, None].astype(np.float64) * inv_freq[None, :]
    cosf = np.ascontiguousarray(np.cos(ang).astype(np.float32))
    sinf = np.ascontiguousarray(np.sin(ang).astype(np.float32))

    in_maps = []
    for c in range(N_CORES):
        b, ch = divmod(c, NQT)
        qs = ch * CH
        cq = np.ascontiguousarray(np.tile(cosf[qs:qs + CH, :].T, (NQT, 1)))
        sq = np.ascontiguousarray(np.tile(sinf[qs:qs + CH, :].T, (NQT, 1)))
        # keys are reordered so this core's own q-chunk pairs come first
        # (attention is permutation-invariant over keys); the rope tables
        # below follow the same order
        myp = [2 * ch, 2 * ch + 1]
        rest = [p for p in range(NKT // 2) if p not in myp]
        key_perm = np.concatenate(
            [np.arange(256 * p, 256 * (p + 1)) for p in myp + rest]
        )
        in_maps.append({
            "hst4": np.ascontiguousarray(hst_all[b][rest]),
            "hsqt": np.ascontiguousarray(hst_all[b][myp]),
            "wqa_t": wqa_t,
            "wqb_t": wqb_t,
            "wqr_t": wqr_t,
            "wkva_t": wkva_t,
            "kup_t": kup_t,
            "vup_t": vup_t,
            "wo_t": wo_t,
            "bqn_v": bqn,
            "bqr_v": bqr_perm,
            "bkn_v": bkn,
            "bvv_v": bvv,
            "ones_in": ones_in,
            "ck_tab": np.ascontiguousarray(cosf[key_perm]),
            "sk_tab": np.ascontiguousarray(sinf[key_perm]),
            "cq_tab": cq,
            "sq_tab": sq,
        })
    return in_maps


def kernel(**inputs) -> np.ndarray:
    nc = _get_nc()
    in_maps = _prep_in_maps(inputs)
    res = run_bass_kernel_spmd(nc, in_maps, core_ids=list(range(N_CORES)))
    out = np.empty((B, S, D), dtype=np.float32)
    for c in range(N_CORES):
        b, ch = divmod(c, NQT)
        out[b, ch * CH:(ch + 1) * CH, :] = res.results[c]["out_c"]
    return out
